# revision 24
# baseline (speedup 1.0000x reference)
"""Multi-head attention (B=2, S=2048, D=512, H=8, E=64) on 8 TRN2 NeuronCores.

Sharding (data parallel over batch x query-blocks):
  core c -> batch b = c // 4, query rows [512*(c%4), 512*(c%4+1)).
Each core projects K/V for all 2048 keys of its batch (work duplicated
across the 4 cores of a batch -- no collectives needed), computes all 8
heads of attention for its 512 query rows, applies the output projection
and writes its [512, 512] block of the output.

v2 changes vs the first working kernel:
  - all inputs arrive pre-packed in bf16 from the host (x, weights, and
    keep = 1-mask in the doubled stream layout), so there are no on-device
    CAST / gpsimd mask conversions and HBM traffic is halved.
  - softmax normalization is DMA-free: the two per-parity row-sum rows are
    copied out of PSUM, reciprocal'd on DVE as a [2, 512] tile, and
    broadcast to 128 partitions with a tiny PE matmul against a constant
    0/1 selector -- no DRAM round trips, so the PE never stalls at pair
    boundaries and HAM stays warm.
  - score matmuls are 4-way tiled on the PE (row tiling over the two
    head parities x column tiling over the two 64-key halves of each
    chunk), doubling score throughput vs the 2-way version.
  - the V bias is folded into an effective output bias on the host
    (out = o@Wo + (bv_cat@Wo + bo)), so V projection evac is a plain copy.
  - the keep-mask multiply is split DVE / GpSimd to keep DVE off the
    critical path.
"""

import sys

import numpy as np

if "/opt/trn_rl_repo" not in sys.path:
    sys.path.insert(0, "/opt/trn_rl_repo")

import ml_dtypes

import concourse.bass as bass  # noqa: F401
import concourse.tile as tile
from concourse import bacc, mybir

FP32 = mybir.dt.float32
BF16 = mybir.dt.bfloat16
AF = mybir.ActivationFunctionType
ALU = mybir.AluOpType
BF16NP = ml_dtypes.bfloat16

B, S, D, H, E = 2, 2048, 512, 8, 64
P = 128
QB = 512          # query rows per core
NQC = QB // P     # 4 query chunks
NKC = S // P      # 16 key chunks
NDC = D // P      # 4 contraction chunks over D
NPAIR = H // 2    # 4 head pairs
EV = E + 1        # V columns incl. the ones-column for row sums
# stream items per head-pair: s -> (head parity s%2, key chunk s//2).
# Grouped in 3s to match the [128, 3, 512] PSUM score tiles (3 banks).
NSTREAM = 2 * NKC
GROUPS = [(g, min(3, NSTREAM - g)) for g in range(0, NSTREAM, 3)]

N_CORES = 8


def build_program():
    nc = bacc.Bacc("TRN2", num_devices=N_CORES)

    xt_d = nc.dram_tensor("xt", [D, S], BF16, kind="ExternalInput")      # x[b].T
    xqt_d = nc.dram_tensor("xqt", [D, QB], BF16, kind="ExternalInput")   # x[b, q0:q0+QB].T
    keep_d = nc.dram_tensor("keep", [P, NKC, QB], BF16, kind="ExternalInput")
    wq_d = nc.dram_tensor("wq", [P, NDC, D], BF16, kind="ExternalInput")  # [p, dc, (h e)]
    wk_d = nc.dram_tensor("wk", [P, NDC, D], BF16, kind="ExternalInput")
    wv_d = nc.dram_tensor("wv", [P, NDC, D], BF16, kind="ExternalInput")
    wo_d = nc.dram_tensor("wo", [P, NDC, D], BF16, kind="ExternalInput")  # [p, dc, dout]
    bqk_d = nc.dram_tensor("bqk", [P, 2 * NPAIR], FP32, kind="ExternalInput")
    bo_d = nc.dram_tensor("bo", [1, D], FP32, kind="ExternalInput")  # bo + bv_cat @ Wo
    out_d = nc.dram_tensor("out", [QB, D], FP32, kind="ExternalOutput")

    with tile.TileContext(nc) as tc:
        with (
            tc.tile_pool(name="persist", bufs=1) as persist,
            tc.tile_pool(name="expp", bufs=8) as expp,
            tc.tile_pool(name="small", bufs=4) as small,
            tc.tile_pool(name="psum_s", bufs=2, space="PSUM") as psum_s,
            tc.tile_pool(name="psum_m", bufs=2, space="PSUM") as psum_m,
        ):
            # constant ones row for the reciprocal broadcast matmuls (fp32
            # so the broadcast matmul consumes the reciprocal directly)
            ones1 = persist.tile([1, 64], FP32, tag="ones1")
            nc.vector.memset(ones1[:], 1.0)
            # touch Exp right away so the ACT table set loads during the
            # initial DMA wait instead of delaying the first projection evac
            actwarm = persist.tile([1, 8], FP32, tag="actwarm")
            nc.vector.memset(actwarm[:], 0.0)
            nc.scalar.activation(actwarm[:], actwarm[:], AF.Exp)

            # ---------------- loads ----------------
            # few, large DMAs (each dma_start costs ~650ns of issue time on
            # its HWDGE engine); keep/wo/bob ride the scalar-engine ring so
            # the sync ring drains the matmul-critical tensors first
            wq_sb = persist.tile([P, NDC, D], BF16, tag="wq")
            nc.sync.dma_start(out=wq_sb[:], in_=wq_d[:])
            xqT = persist.tile([P, NDC, QB], BF16, tag="xqT")
            nc.sync.dma_start(
                out=xqT[:], in_=xqt_d[:].rearrange("(dc p) q -> p dc q", p=P)
            )
            bqk_sb = persist.tile([P, 2 * NPAIR], FP32, tag="bqk")
            nc.sync.dma_start(out=bqk_sb[:], in_=bqk_d[:])
            wk_sb = persist.tile([P, NDC, D], BF16, tag="wk")
            nc.sync.dma_start(out=wk_sb[:], in_=wk_d[:])
            xT = persist.tile([P, NDC, S], BF16, tag="xT")
            xt_r = xt_d[:].rearrange("(dc p) s -> p dc s", p=P)
            wv_sb = persist.tile([P, NDC, D], BF16, tag="wv")
            for kb in range(NDC):
                nc.sync.dma_start(
                    out=xT[:, :, kb * QB:(kb + 1) * QB],
                    in_=xt_r[:, :, kb * QB:(kb + 1) * QB],
                )
                if kb == 0:
                    nc.sync.dma_start(out=wv_sb[:], in_=wv_d[:])

            # keep^T = 1 - mask^T (bf16), one copy per key chunk; the mask
            # multiply broadcasts it over the two head parities via a
            # stride-0 middle dim
            keepT = persist.tile([P, NKC, QB], BF16, tag="keepT")
            for kq in range(4):
                nc.scalar.dma_start(
                    out=keepT[:, 4 * kq:4 * (kq + 1), :],
                    in_=keep_d[:, 4 * kq:4 * (kq + 1), :],
                )
            wo_sb = persist.tile([P, NDC, D], BF16, tag="wo")
            nc.scalar.dma_start(out=wo_sb[:], in_=wo_d[:])
            bob = persist.tile([P, D], FP32, tag="bob")
            nc.scalar.dma_start(out=bob[:], in_=bo_d[:].to_broadcast((P, D)))

            in_attention = [False]

            def proj_psum(i):
                # once attention starts, both psum_m slots are held by the
                # running pair's o accumulators -- lazy projections must cycle
                # through the psum_s (score) slots only.
                if in_attention[0] or i % 2 == 1:
                    return psum_s.tile([P, 3, QB], FP32, tag="sc", name="sc")[:, 0, :]
                return psum_m.tile([P, QB], FP32, tag="pm", name="pm")

            # ---------------- projections ----------------
            QT = persist.tile([P, NPAIR, QB], BF16, tag="QT")

            def emit_q_proj(pr):
                ps = proj_psum(pr)
                for dc in range(NDC):
                    nc.tensor.matmul(
                        ps[:],
                        lhsT=wq_sb[:, dc, pr * P:(pr + 1) * P],
                        rhs=xqT[:, dc, :],
                        start=(dc == 0),
                        stop=(dc == NDC - 1),
                    )
                nc.scalar.activation(
                    QT[:, pr, :], ps[:], AF.Identity, bias=bqk_sb[:, pr:pr + 1]
                )

            KT = persist.tile([P, NPAIR, S], BF16, tag="KT")
            Vp = persist.tile([P, NKC, H * EV], BF16, tag="Vp")
            # ones-columns of V (row-sum trick), written once
            nc.vector.memset(
                Vp[:].rearrange("p kc (h w) -> p kc h w", w=EV)[:, :, :, E], 1.0
            )

            def emit_k_proj_kb(pr, kb, evac="act"):
                ps = proj_psum(pr * NDC + kb)
                for dc in range(NDC):
                    nc.tensor.matmul(
                        ps[:],
                        lhsT=wk_sb[:, dc, pr * P:(pr + 1) * P],
                        rhs=xT[:, dc, kb * QB:(kb + 1) * QB],
                        start=(dc == 0),
                        stop=(dc == NDC - 1),
                    )
                if evac == "act":
                    nc.scalar.activation(
                        KT[:, pr, kb * QB:(kb + 1) * QB], ps[:], AF.Identity,
                        bias=bqk_sb[:, NPAIR + pr:NPAIR + pr + 1],
                    )
                else:
                    nc.vector.tensor_scalar_add(
                        KT[:, pr, kb * QB:(kb + 1) * QB], ps[:],
                        bqk_sb[:, NPAIR + pr:NPAIR + pr + 1],
                    )

            def emit_v_proj(kc):
                ps = proj_psum(kc)
                for dc in range(NDC):
                    nc.tensor.matmul(
                        ps[:],
                        lhsT=xT[:, dc, kc * P:(kc + 1) * P],
                        rhs=wv_sb[:, dc, :],
                        start=(dc == 0),
                        stop=(dc == NDC - 1),
                    )
                nc.vector.tensor_copy(
                    Vp[:, kc, :].rearrange("p (h w) -> p h w", w=EV)[:, :, 0:E],
                    ps[:].rearrange("p (h e) -> p h e", e=E),
                )

            # pre-attention: only what attention group 0 strictly needs;
            # K chunks kb2/kb3 of pair 0 move into pair 0's stream
            for pr in range(NPAIR):
                emit_q_proj(pr)
            emit_k_proj_kb(0, 0)
            for kc in range(4):
                emit_v_proj(kc)
            emit_k_proj_kb(0, 1)
            for kc in range(4, 8):
                emit_v_proj(kc)

            # ---------------- attention ----------------
            # o_all^T accumulated as [(d % 128), d // 128, q] with d = h*64+e.
            # Scores run 4-way concurrent on the PE: row tiling over head
            # parity (rows 0-63 / 64-127) x column tiling over the two 64-key
            # halves of each chunk (PSUM partitions 0-63 / 64-127).
            oT = persist.tile([P, NDC, QB], BF16, tag="oT")
            in_attention[0] = True

            def norm_stash(pr, o_ps):
                # unnormalized o^T + raw row sums straight out of PSUM; the
                # [1, 512] sum rows each land in partition 0 of a small tile
                sums = []
                for par in range(2):
                    off = par * 64
                    nc.vector.tensor_copy(
                        out=oT[off:off + 64, pr, :], in_=o_ps[par][0:64, :]
                    )
                    srow = small.tile([1, QB], FP32, tag=f"srow{par}")
                    nc.vector.tensor_copy(out=srow[:], in_=o_ps[par][E:E + 1, :])
                    sums.append(srow)
                return sums

            def norm_recip(pr, sums):
                recs = []
                for par in range(2):
                    r32 = small.tile([1, QB], FP32, tag=f"r32{par}")
                    nc.vector.reciprocal_approx_fast(out=r32[:], in_=sums[par][:])
                    recs.append(r32)
                return recs

            def norm_bcast(pr, recs, from_m=False):
                # broadcast recs[par] over partitions [64*par, 64*par+64)
                # with two tiny col-tiled fp32 matmuls against a ones row.
                # In the tail the score-psum bufs hold output accumulations,
                # so the broadcast uses a freed o-accumulator slot instead.
                if from_m:
                    psb = psum_m.tile([P, QB], FP32, tag="pm", name="psb")
                else:
                    psb = psum_s.tile([P, 3, QB], FP32, tag="sc", name="psb")[:, 0, :]
                for par in range(2):
                    off = par * 64
                    nc.tensor.matmul(
                        psb[off:off + 64, :], lhsT=ones1[:],
                        rhs=recs[par][:],
                        start=True, stop=True,
                    )
                nc.vector.tensor_tensor(
                    oT[:, pr, :], oT[:, pr, :], psb[:], ALU.mult
                )

            def emit_pv_item(o_ps, pr, s, ex, j):
                par, kc = s % 2, s // 2
                h = 2 * pr + par
                nc.tensor.matmul(
                    o_ps[par][0:EV, :],
                    lhsT=Vp[:, kc, h * EV:(h + 1) * EV],
                    rhs=ex[:, j, :],
                    start=(s < 2),
                    stop=(s >= NSTREAM - 2),
                )

            # global software pipeline: the PV matmuls for score group g are
            # emitted after the scores of group g+2 -- carried ACROSS pair
            # boundaries so pair ends never degenerate into a serial
            # scores->exp->mask->PV chain. Each queue entry is one group.
            pvq = []  # (o_ps, pr, g0, glen, ex, is_last_of_pair)

            def drain_one():
                o_ps_, pr_, g0_, glen_, ex_, last_ = pvq.pop(0)
                for j in range(glen_):
                    emit_pv_item(o_ps_, pr_, g0_ + j, ex_, j)
                if last_:
                    pending.append((pr_, norm_stash(pr_, o_ps_)))

            pending = []  # (pr, sums) awaiting norm, at most one entry
            for pr in range(NPAIR):
                o_ps0 = psum_m.tile([P, QB], FP32, tag="pm", name="o0")
                o_ps1 = psum_m.tile([P, QB], FP32, tag="pm", name="o1")
                o_ps = (o_ps0, o_ps1)
                for gi, (g0, glen) in enumerate(GROUPS):
                    if pr == 0 and gi < 2:
                        emit_k_proj_kb(0, gi + 2)
                    if pr == 0 and gi < 8:
                        emit_v_proj(gi + 8)
                    if pr < NPAIR - 1 and 6 <= gi < 10:
                        emit_k_proj_kb(
                            pr + 1, gi - 6, evac="act" if gi < 8 else "dve"
                        )
                    if pending:
                        ppr, sums = pending[0]
                        if gi == 3:
                            recs = norm_recip(ppr, sums)
                        elif gi == 5:
                            norm_bcast(ppr, recs)
                            pending.pop(0)
                    sc = psum_s.tile([P, 3, QB], FP32, tag="sc", name="sc")
                    for j in range(glen):
                        s = g0 + j
                        par, kc = s % 2, s // 2
                        rt = par * 64
                        for half in range(2):
                            co = half * 64
                            nc.tensor.matmul(
                                sc[co:co + 64, j, :],
                                lhsT=KT[rt:rt + 64, pr,
                                        kc * P + co:kc * P + co + 64],
                                rhs=QT[rt:rt + 64, pr, :],
                                start=True,
                                stop=True,
                            )
                    if len(pvq) >= 2:
                        drain_one()
                    ex = expp.tile([P, 3, QB], BF16, tag="ex")
                    nc.scalar.activation(
                        ex[:, 0:glen, :], sc[:, 0:glen, :], AF.Exp, scale=0.125
                    )
                    kc0 = g0 // 2
                    if g0 % 2 == 0:
                        nc.vector.tensor_tensor(
                            ex[:, 0:min(2, glen), :], ex[:, 0:min(2, glen), :],
                            keepT[:, kc0:kc0 + 1, :].to_broadcast(
                                (P, min(2, glen), QB)
                            ),
                            ALU.mult,
                        )
                        if glen > 2:
                            nc.vector.tensor_tensor(
                                ex[:, 2:3, :], ex[:, 2:3, :],
                                keepT[:, kc0 + 1:kc0 + 2, :], ALU.mult,
                            )
                    else:
                        nc.vector.tensor_tensor(
                            ex[:, 0:1, :], ex[:, 0:1, :],
                            keepT[:, kc0:kc0 + 1, :], ALU.mult,
                        )
                        if glen > 1:
                            nc.vector.tensor_tensor(
                                ex[:, 1:glen, :], ex[:, 1:glen, :],
                                keepT[:, kc0 + 1:kc0 + 2, :].to_broadcast(
                                    (P, glen - 1, QB)
                                ),
                                ALU.mult,
                            )
                    pvq.append((o_ps, pr, g0, glen, ex, gi == len(GROUPS) - 1))

            # tail: the output projection for pairs 0-2 accumulates into
            # PSUM while the last two PV groups drain and pair 3 normalizes,
            # then each query chunk is finished and streamed out

            def oproj_mms(ps, qc, prs, start):
                for i, pr in enumerate(prs):
                    nc.tensor.matmul(
                        ps[:],
                        lhsT=oT[:, pr, qc * P:(qc + 1) * P],
                        rhs=wo_sb[:, pr, :],
                        start=start and i == 0,
                        stop=(pr == NPAIR - 1),
                    )

            def oproj_finish(ps, qc):
                oq = small.tile([P, QB], FP32, tag="oq")
                nc.vector.tensor_tensor(oq[:], ps[:], bob[:], ALU.add)
                nc.sync.dma_start(
                    out=out_d[qc * P:(qc + 1) * P, :], in_=oq[:]
                )

            psA = psum_s.tile([P, 3, QB], FP32, tag="sc", name="scp")[:, 0, :]
            oproj_mms(psA, 0, [0, 1, 2], start=True)
            psB = psum_s.tile([P, 3, QB], FP32, tag="sc", name="scp")[:, 0, :]
            oproj_mms(psB, 1, [0, 1, 2], start=True)
            while pvq:
                drain_one()
            ppr, sums = pending.pop(0)
            recs = norm_recip(ppr, sums)
            norm_bcast(ppr, recs, from_m=True)
            oproj_mms(psA, 0, [3], start=False)
            oproj_finish(psA, 0)
            oproj_mms(psB, 1, [3], start=False)
            oproj_finish(psB, 1)
            for qc in (2, 3):
                ps = psum_s.tile([P, 3, QB], FP32, tag="sc", name="scp")[:, 0, :]
                oproj_mms(ps, qc, [0, 1, 2, 3], start=True)
                oproj_finish(ps, qc)

    nc.finalize()
    return nc


_NC = None


def get_program():
    global _NC
    if _NC is None:
        _NC = build_program()
    return _NC


def make_in_maps(inputs):
    x = np.asarray(inputs["x"], dtype=np.float32)
    mask = np.asarray(inputs["attention_mask"], dtype=np.int32)
    Wq = np.asarray(inputs["Wq"], dtype=np.float32)
    Wk = np.asarray(inputs["Wk"], dtype=np.float32)
    Wv = np.asarray(inputs["Wv"], dtype=np.float32)
    Wo = np.asarray(inputs["Wo"], dtype=np.float32)
    bq = np.asarray(inputs["bq"], dtype=np.float32).reshape(-1)
    bk = np.asarray(inputs["bk"], dtype=np.float32).reshape(-1)
    bv = np.asarray(inputs["bv"], dtype=np.float32).reshape(-1)
    bo = np.asarray(inputs["bo"], dtype=np.float32).reshape(-1)

    def pack_w(W):  # [H, D, E] -> [p, dc, h*64+e]
        return np.ascontiguousarray(
            W.reshape(H, NDC, P, E).transpose(2, 1, 0, 3).reshape(P, NDC, D)
        ).astype(BF16NP)

    wq_r, wk_r, wv_r = pack_w(Wq), pack_w(Wk), pack_w(Wv)
    wo_r = np.ascontiguousarray(Wo.reshape(NDC, P, D).transpose(1, 0, 2)).astype(
        BF16NP
    )
    bqk = np.empty((P, 2 * NPAIR), np.float32)
    bqk[:, 0:NPAIR] = bq.reshape(NPAIR, P).T
    bqk[:, NPAIR:] = bk.reshape(NPAIR, P).T
    # fold the V bias through the output projection: o@Wo + (bv_cat@Wo + bo)
    bo_eff = (bv @ Wo + bo).astype(np.float32).reshape(1, -1)

    xt_all = [np.ascontiguousarray(x[b].T).astype(BF16NP) for b in range(B)]
    keep_all = (1 - mask).astype(BF16NP)  # [B, Sq, Sk]
    in_maps = []
    for c in range(N_CORES):
        b, q0 = c // 4, QB * (c % 4)
        # keep^T slice [2048 keys, 512 q] -> [128, kc, 512]
        kT = np.ascontiguousarray(keep_all[b, q0:q0 + QB, :].T)  # [S, QB]
        kT = np.ascontiguousarray(kT.reshape(NKC, P, QB).transpose(1, 0, 2))
        in_maps.append({
            "xt": xt_all[b],
            "xqt": np.ascontiguousarray(xt_all[b][:, q0:q0 + QB]),
            "keep": kT,
            "wq": wq_r, "wk": wk_r, "wv": wv_r, "wo": wo_r,
            "bqk": bqk, "bo": bo_eff,
        })
    return in_maps


def assemble(results):
    out = np.empty((B, S, D), np.float32)
    for c in range(N_CORES):
        b, q0 = c // 4, QB * (c % 4)
        out[b, q0:q0 + QB, :] = results[c]["out"]
    return out


def run(inputs, **kwargs):
    from concourse.bass_utils import run_bass_kernel_spmd

    nc = get_program()
    in_maps = make_in_maps(inputs)
    return run_bass_kernel_spmd(nc, in_maps, list(range(N_CORES)), **kwargs)


def kernel(**inputs) -> np.ndarray:
    res = run(inputs)
    return assemble(res.results)


if __name__ == "__main__":
    nc = build_program()
    print("program built ok")


# revision 25
# speedup vs baseline: 1.0637x; 1.0637x over previous
"""Multi-head attention (B=2, S=2048, D=512, H=8, E=64) on 8 TRN2 NeuronCores.

Sharding (data parallel over batch x query-blocks):
  core c -> batch b = c // 4, query rows [512*(c%4), 512*(c%4+1)).
Each core projects K/V for all 2048 keys of its batch (work duplicated
across the 4 cores of a batch -- no collectives needed), computes all 8
heads of attention for its 512 query rows, applies the output projection
and writes its [512, 512] block of the output.

v2 changes vs the first working kernel:
  - all inputs arrive pre-packed in bf16 from the host (x, weights, and
    keep = 1-mask in the doubled stream layout), so there are no on-device
    CAST / gpsimd mask conversions and HBM traffic is halved.
  - softmax normalization is DMA-free: the two per-parity row-sum rows are
    copied out of PSUM, reciprocal'd on DVE as a [2, 512] tile, and
    broadcast to 128 partitions with a tiny PE matmul against a constant
    0/1 selector -- no DRAM round trips, so the PE never stalls at pair
    boundaries and HAM stays warm.
  - score matmuls are 4-way tiled on the PE (row tiling over the two
    head parities x column tiling over the two 64-key halves of each
    chunk), doubling score throughput vs the 2-way version.
  - the V bias is folded into an effective output bias on the host
    (out = o@Wo + (bv_cat@Wo + bo)), so V projection evac is a plain copy.
  - the keep-mask multiply is split DVE / GpSimd to keep DVE off the
    critical path.
"""

import sys

import numpy as np

if "/opt/trn_rl_repo" not in sys.path:
    sys.path.insert(0, "/opt/trn_rl_repo")

import ml_dtypes

import concourse.bass as bass  # noqa: F401
import concourse.tile as tile
from concourse import bacc, mybir

FP32 = mybir.dt.float32
BF16 = mybir.dt.bfloat16
AF = mybir.ActivationFunctionType
ALU = mybir.AluOpType
BF16NP = ml_dtypes.bfloat16

B, S, D, H, E = 2, 2048, 512, 8, 64
P = 128
QB = 512          # query rows per core
NQC = QB // P     # 4 query chunks
NKC = S // P      # 16 key chunks
NDC = D // P      # 4 contraction chunks over D
NPAIR = H // 2    # 4 head pairs
EV = E + 1        # V columns incl. the ones-column for row sums
# stream items per head-pair: s -> (head parity s%2, key chunk s//2).
# Grouped in 3s to match the [128, 3, 512] PSUM score tiles (3 banks).
NSTREAM = 2 * NKC
GROUPS = [(g, min(3, NSTREAM - g)) for g in range(0, NSTREAM, 3)]

N_CORES = 8


def build_program():
    nc = bacc.Bacc("TRN2", num_devices=N_CORES)

    xt_d = nc.dram_tensor("xt", [D, S], BF16, kind="ExternalInput")      # x[b].T
    xqt_d = nc.dram_tensor("xqt", [D, QB], BF16, kind="ExternalInput")   # x[b, q0:q0+QB].T
    keep_d = nc.dram_tensor("keep", [P, NKC, QB], BF16, kind="ExternalInput")
    wq_d = nc.dram_tensor("wq", [P, NDC, D], BF16, kind="ExternalInput")  # [p, dc, (h e)]
    wk_d = nc.dram_tensor("wk", [P, NDC, D], BF16, kind="ExternalInput")
    wv_d = nc.dram_tensor("wv", [P, NDC, D], BF16, kind="ExternalInput")
    wo_d = nc.dram_tensor("wo", [P, NDC, D], BF16, kind="ExternalInput")  # [p, dc, dout]
    bqk_d = nc.dram_tensor("bqk", [P, 2 * NPAIR], FP32, kind="ExternalInput")
    bo_d = nc.dram_tensor("bo", [1, D], FP32, kind="ExternalInput")  # bo + bv_cat @ Wo
    out_d = nc.dram_tensor("out", [QB, D], FP32, kind="ExternalOutput")

    with tile.TileContext(nc) as tc:
        with (
            tc.tile_pool(name="persist", bufs=1) as persist,
            tc.tile_pool(name="expp", bufs=8) as expp,
            tc.tile_pool(name="small", bufs=4) as small,
            tc.tile_pool(name="psum_s", bufs=2, space="PSUM") as psum_s,
            tc.tile_pool(name="psum_m", bufs=2, space="PSUM") as psum_m,
        ):
            # constant ones row for the reciprocal broadcast matmuls
            ones1 = persist.tile([1, 64], BF16, tag="ones1")
            nc.vector.memset(ones1[:], 1.0)
            # touch Exp right away so the ACT table set loads during the
            # initial DMA wait instead of delaying the first projection evac
            actwarm = persist.tile([1, 8], FP32, tag="actwarm")
            nc.vector.memset(actwarm[:], 0.0)
            nc.scalar.activation(actwarm[:], actwarm[:], AF.Exp)

            # ---------------- loads ----------------
            # few, large DMAs (each dma_start costs ~650ns of issue time on
            # its HWDGE engine); keep/wo/bob ride the scalar-engine ring so
            # the sync ring drains the matmul-critical tensors first
            wq_sb = persist.tile([P, NDC, D], BF16, tag="wq")
            nc.sync.dma_start(out=wq_sb[:], in_=wq_d[:])
            xqT = persist.tile([P, NDC, QB], BF16, tag="xqT")
            nc.sync.dma_start(
                out=xqT[:], in_=xqt_d[:].rearrange("(dc p) q -> p dc q", p=P)
            )
            bqk_sb = persist.tile([P, 2 * NPAIR], FP32, tag="bqk")
            nc.sync.dma_start(out=bqk_sb[:], in_=bqk_d[:])
            wk_sb = persist.tile([P, NDC, D], BF16, tag="wk")
            nc.sync.dma_start(out=wk_sb[:], in_=wk_d[:])
            xT = persist.tile([P, NDC, S], BF16, tag="xT")
            xt_r = xt_d[:].rearrange("(dc p) s -> p dc s", p=P)
            wv_sb = persist.tile([P, NDC, D], BF16, tag="wv")
            for kb in range(NDC):
                nc.sync.dma_start(
                    out=xT[:, :, kb * QB:(kb + 1) * QB],
                    in_=xt_r[:, :, kb * QB:(kb + 1) * QB],
                )
                if kb == 0:
                    nc.sync.dma_start(out=wv_sb[:], in_=wv_d[:])

            # keep^T = 1 - mask^T (bf16), one copy per key chunk; the mask
            # multiply broadcasts it over the two head parities via a
            # stride-0 middle dim
            keepT = persist.tile([P, NKC, QB], BF16, tag="keepT")
            for kq in range(4):
                nc.scalar.dma_start(
                    out=keepT[:, 4 * kq:4 * (kq + 1), :],
                    in_=keep_d[:, 4 * kq:4 * (kq + 1), :],
                )
            wo_sb = persist.tile([P, NDC, D], BF16, tag="wo")
            nc.scalar.dma_start(out=wo_sb[:], in_=wo_d[:])
            bob = persist.tile([P, D], FP32, tag="bob")
            nc.scalar.dma_start(out=bob[:], in_=bo_d[:].to_broadcast((P, D)))

            in_attention = [False]

            def proj_psum(i):
                # once attention starts, both psum_m slots are held by the
                # running pair's o accumulators -- lazy projections must cycle
                # through the psum_s (score) slots only.
                if in_attention[0] or i % 2 == 1:
                    return psum_s.tile([P, 3, QB], FP32, tag="sc", name="sc")[:, 0, :]
                return psum_m.tile([P, QB], FP32, tag="pm", name="pm")

            # ---------------- projections ----------------
            QT = persist.tile([P, NPAIR, QB], BF16, tag="QT")

            def emit_q_proj(pr):
                ps = proj_psum(pr)
                for dc in range(NDC):
                    nc.tensor.matmul(
                        ps[:],
                        lhsT=wq_sb[:, dc, pr * P:(pr + 1) * P],
                        rhs=xqT[:, dc, :],
                        start=(dc == 0),
                        stop=(dc == NDC - 1),
                    )
                nc.scalar.activation(
                    QT[:, pr, :], ps[:], AF.Identity, bias=bqk_sb[:, pr:pr + 1]
                )

            KT = persist.tile([P, NPAIR, S], BF16, tag="KT")
            Vp = persist.tile([P, NKC, H * EV], BF16, tag="Vp")
            # ones-columns of V (row-sum trick), written once
            nc.vector.memset(
                Vp[:].rearrange("p kc (h w) -> p kc h w", w=EV)[:, :, :, E], 1.0
            )

            def emit_k_proj_kb(pr, kb, evac="act"):
                ps = proj_psum(pr * NDC + kb)
                for dc in range(NDC):
                    nc.tensor.matmul(
                        ps[:],
                        lhsT=wk_sb[:, dc, pr * P:(pr + 1) * P],
                        rhs=xT[:, dc, kb * QB:(kb + 1) * QB],
                        start=(dc == 0),
                        stop=(dc == NDC - 1),
                    )
                if evac == "act":
                    nc.scalar.activation(
                        KT[:, pr, kb * QB:(kb + 1) * QB], ps[:], AF.Identity,
                        bias=bqk_sb[:, NPAIR + pr:NPAIR + pr + 1],
                    )
                else:
                    nc.vector.tensor_scalar_add(
                        KT[:, pr, kb * QB:(kb + 1) * QB], ps[:],
                        bqk_sb[:, NPAIR + pr:NPAIR + pr + 1],
                    )

            def emit_v_proj(kc):
                ps = proj_psum(kc)
                for dc in range(NDC):
                    nc.tensor.matmul(
                        ps[:],
                        lhsT=xT[:, dc, kc * P:(kc + 1) * P],
                        rhs=wv_sb[:, dc, :],
                        start=(dc == 0),
                        stop=(dc == NDC - 1),
                    )
                nc.vector.tensor_copy(
                    Vp[:, kc, :].rearrange("p (h w) -> p h w", w=EV)[:, :, 0:E],
                    ps[:].rearrange("p (h e) -> p h e", e=E),
                )

            # pre-attention: only what attention group 0 strictly needs;
            # K chunks kb2/kb3 of pair 0 move into pair 0's stream
            for pr in range(NPAIR):
                emit_q_proj(pr)
            emit_k_proj_kb(0, 0)
            for kc in range(4):
                emit_v_proj(kc)
            emit_k_proj_kb(0, 1)
            for kc in range(4, 8):
                emit_v_proj(kc)

            # ---------------- attention ----------------
            # o_all^T accumulated as [(d % 128), d // 128, q] with d = h*64+e.
            # Scores run 4-way concurrent on the PE: row tiling over head
            # parity (rows 0-63 / 64-127) x column tiling over the two 64-key
            # halves of each chunk (PSUM partitions 0-63 / 64-127).
            oT = persist.tile([P, NDC, QB], BF16, tag="oT")
            in_attention[0] = True

            def norm_stash(pr, o_ps):
                # unnormalized o^T + raw row sums straight out of PSUM; the
                # [1, 512] sum rows each land in partition 0 of a small tile
                sums = []
                for par in range(2):
                    off = par * 64
                    nc.vector.tensor_copy(
                        out=oT[off:off + 64, pr, :], in_=o_ps[par][0:64, :]
                    )
                    srow = small.tile([1, QB], FP32, tag=f"srow{par}")
                    nc.vector.tensor_copy(out=srow[:], in_=o_ps[par][E:E + 1, :])
                    sums.append(srow)
                return sums

            def norm_recip(pr, sums):
                recs = []
                for par in range(2):
                    r32 = small.tile([1, QB], FP32, tag=f"r32{par}")
                    nc.vector.reciprocal_approx_fast(out=r32[:], in_=sums[par][:])
                    recs.append(r32)
                return recs

            def norm_cast(pr, recs):
                recb2 = small.tile([1, 2, QB], BF16, tag="recb2")
                nc.vector.tensor_copy(out=recb2[:, 0, :], in_=recs[0][:])
                nc.vector.tensor_copy(out=recb2[:, 1, :], in_=recs[1][:])
                return recb2

            def norm_bcast(pr, recb2, from_m=False):
                # broadcast recb2[0, par] over partitions [64*par, 64*par+64)
                # with two tiny col-tiled bf16 matmuls against a ones row.
                # In the tail the score-psum bufs hold output accumulations,
                # so the broadcast uses a freed o-accumulator slot instead.
                if from_m:
                    psb = psum_m.tile([P, QB], FP32, tag="pm", name="psb")
                else:
                    psb = psum_s.tile([P, 3, QB], FP32, tag="sc", name="psb")[:, 0, :]
                for par in range(2):
                    off = par * 64
                    nc.tensor.matmul(
                        psb[off:off + 64, :], lhsT=ones1[:],
                        rhs=recb2[:, par, :],
                        start=True, stop=True,
                    )
                nc.vector.tensor_tensor(
                    oT[:, pr, :], oT[:, pr, :], psb[:], ALU.mult
                )

            def emit_pv_item(o_ps, pr, s, ex, j):
                par, kc = s % 2, s // 2
                h = 2 * pr + par
                nc.tensor.matmul(
                    o_ps[par][0:EV, :],
                    lhsT=Vp[:, kc, h * EV:(h + 1) * EV],
                    rhs=ex[:, j, :],
                    start=(s < 2),
                    stop=(s >= NSTREAM - 2),
                )

            # global software pipeline: the PV matmuls for score group g are
            # emitted after the scores of group g+2 -- carried ACROSS pair
            # boundaries so pair ends never degenerate into a serial
            # scores->exp->mask->PV chain. Each queue entry is one group.
            pvq = []  # (o_ps, pr, g0, glen, ex, is_last_of_pair)

            def drain_one():
                o_ps_, pr_, g0_, glen_, ex_, last_ = pvq.pop(0)
                for j in range(glen_):
                    emit_pv_item(o_ps_, pr_, g0_ + j, ex_, j)
                if last_:
                    pending.append((pr_, norm_stash(pr_, o_ps_)))

            pending = []  # (pr, sums) awaiting norm, at most one entry
            for pr in range(NPAIR):
                o_ps0 = psum_m.tile([P, QB], FP32, tag="pm", name="o0")
                o_ps1 = psum_m.tile([P, QB], FP32, tag="pm", name="o1")
                o_ps = (o_ps0, o_ps1)
                for gi, (g0, glen) in enumerate(GROUPS):
                    if pr == 0 and gi < 2:
                        emit_k_proj_kb(0, gi + 2)
                    if pr == 0 and gi < 8:
                        emit_v_proj(gi + 8)
                    if pr < NPAIR - 1 and 6 <= gi < 10:
                        emit_k_proj_kb(
                            pr + 1, gi - 6, evac="act" if gi < 8 else "dve"
                        )
                    if pending:
                        ppr, sums = pending[0]
                        if gi == 3:
                            recs = norm_recip(ppr, sums)
                        elif gi == 4:
                            recb2 = norm_cast(ppr, recs)
                        elif gi == 6:
                            norm_bcast(ppr, recb2)
                            pending.pop(0)
                    sc = psum_s.tile([P, 3, QB], FP32, tag="sc", name="sc")
                    for j in range(glen):
                        s = g0 + j
                        par, kc = s % 2, s // 2
                        rt = par * 64
                        for half in range(2):
                            co = half * 64
                            nc.tensor.matmul(
                                sc[co:co + 64, j, :],
                                lhsT=KT[rt:rt + 64, pr,
                                        kc * P + co:kc * P + co + 64],
                                rhs=QT[rt:rt + 64, pr, :],
                                start=True,
                                stop=True,
                            )
                    if len(pvq) >= 2:
                        drain_one()
                    ex = expp.tile([P, 3, QB], BF16, tag="ex")
                    nc.scalar.activation(
                        ex[:, 0:glen, :], sc[:, 0:glen, :], AF.Exp, scale=0.125
                    )
                    kc0 = g0 // 2
                    if g0 % 2 == 0:
                        nc.vector.tensor_tensor(
                            ex[:, 0:min(2, glen), :], ex[:, 0:min(2, glen), :],
                            keepT[:, kc0:kc0 + 1, :].to_broadcast(
                                (P, min(2, glen), QB)
                            ),
                            ALU.mult,
                        )
                        if glen > 2:
                            nc.vector.tensor_tensor(
                                ex[:, 2:3, :], ex[:, 2:3, :],
                                keepT[:, kc0 + 1:kc0 + 2, :], ALU.mult,
                            )
                    else:
                        nc.vector.tensor_tensor(
                            ex[:, 0:1, :], ex[:, 0:1, :],
                            keepT[:, kc0:kc0 + 1, :], ALU.mult,
                        )
                        if glen > 1:
                            nc.vector.tensor_tensor(
                                ex[:, 1:glen, :], ex[:, 1:glen, :],
                                keepT[:, kc0 + 1:kc0 + 2, :].to_broadcast(
                                    (P, glen - 1, QB)
                                ),
                                ALU.mult,
                            )
                    pvq.append((o_ps, pr, g0, glen, ex, gi == len(GROUPS) - 1))

            # tail: the output projection for pairs 0-2 accumulates into
            # PSUM while the last two PV groups drain and pair 3 normalizes,
            # then each query chunk is finished and streamed out

            def oproj_mms(ps, qc, prs, start):
                for i, pr in enumerate(prs):
                    nc.tensor.matmul(
                        ps[:],
                        lhsT=oT[:, pr, qc * P:(qc + 1) * P],
                        rhs=wo_sb[:, pr, :],
                        start=start and i == 0,
                        stop=(pr == NPAIR - 1),
                    )

            def oproj_finish(ps, qc):
                oq = small.tile([P, QB], FP32, tag="oq")
                nc.vector.tensor_tensor(oq[:], ps[:], bob[:], ALU.add)
                nc.sync.dma_start(
                    out=out_d[qc * P:(qc + 1) * P, :], in_=oq[:]
                )

            psA = psum_s.tile([P, 3, QB], FP32, tag="sc", name="scp")[:, 0, :]
            oproj_mms(psA, 0, [0, 1, 2], start=True)
            psB = psum_s.tile([P, 3, QB], FP32, tag="sc", name="scp")[:, 0, :]
            oproj_mms(psB, 1, [0, 1, 2], start=True)
            while pvq:
                drain_one()
            ppr, sums = pending.pop(0)
            recs = norm_recip(ppr, sums)
            recb2 = norm_cast(ppr, recs)
            norm_bcast(ppr, recb2, from_m=True)
            oproj_mms(psA, 0, [3], start=False)
            oproj_finish(psA, 0)
            oproj_mms(psB, 1, [3], start=False)
            oproj_finish(psB, 1)
            for qc in (2, 3):
                ps = psum_s.tile([P, 3, QB], FP32, tag="sc", name="scp")[:, 0, :]
                oproj_mms(ps, qc, [0, 1, 2, 3], start=True)
                oproj_finish(ps, qc)

    nc.finalize()
    return nc


_NC = None


def get_program():
    global _NC
    if _NC is None:
        _NC = build_program()
    return _NC


def make_in_maps(inputs):
    x = np.asarray(inputs["x"], dtype=np.float32)
    mask = np.asarray(inputs["attention_mask"], dtype=np.int32)
    Wq = np.asarray(inputs["Wq"], dtype=np.float32)
    Wk = np.asarray(inputs["Wk"], dtype=np.float32)
    Wv = np.asarray(inputs["Wv"], dtype=np.float32)
    Wo = np.asarray(inputs["Wo"], dtype=np.float32)
    bq = np.asarray(inputs["bq"], dtype=np.float32).reshape(-1)
    bk = np.asarray(inputs["bk"], dtype=np.float32).reshape(-1)
    bv = np.asarray(inputs["bv"], dtype=np.float32).reshape(-1)
    bo = np.asarray(inputs["bo"], dtype=np.float32).reshape(-1)

    def pack_w(W):  # [H, D, E] -> [p, dc, h*64+e]
        return np.ascontiguousarray(
            W.reshape(H, NDC, P, E).transpose(2, 1, 0, 3).reshape(P, NDC, D)
        ).astype(BF16NP)

    wq_r, wk_r, wv_r = pack_w(Wq), pack_w(Wk), pack_w(Wv)
    wo_r = np.ascontiguousarray(Wo.reshape(NDC, P, D).transpose(1, 0, 2)).astype(
        BF16NP
    )
    bqk = np.empty((P, 2 * NPAIR), np.float32)
    bqk[:, 0:NPAIR] = bq.reshape(NPAIR, P).T
    bqk[:, NPAIR:] = bk.reshape(NPAIR, P).T
    # fold the V bias through the output projection: o@Wo + (bv_cat@Wo + bo)
    bo_eff = (bv @ Wo + bo).astype(np.float32).reshape(1, -1)

    xt_all = [np.ascontiguousarray(x[b].T).astype(BF16NP) for b in range(B)]
    keep_all = (1 - mask).astype(BF16NP)  # [B, Sq, Sk]
    in_maps = []
    for c in range(N_CORES):
        b, q0 = c // 4, QB * (c % 4)
        # keep^T slice [2048 keys, 512 q] -> [128, kc, 512]
        kT = np.ascontiguousarray(keep_all[b, q0:q0 + QB, :].T)  # [S, QB]
        kT = np.ascontiguousarray(kT.reshape(NKC, P, QB).transpose(1, 0, 2))
        in_maps.append({
            "xt": xt_all[b],
            "xqt": np.ascontiguousarray(xt_all[b][:, q0:q0 + QB]),
            "keep": kT,
            "wq": wq_r, "wk": wk_r, "wv": wv_r, "wo": wo_r,
            "bqk": bqk, "bo": bo_eff,
        })
    return in_maps


def assemble(results):
    out = np.empty((B, S, D), np.float32)
    for c in range(N_CORES):
        b, q0 = c // 4, QB * (c % 4)
        out[b, q0:q0 + QB, :] = results[c]["out"]
    return out


def run(inputs, **kwargs):
    from concourse.bass_utils import run_bass_kernel_spmd

    nc = get_program()
    in_maps = make_in_maps(inputs)
    return run_bass_kernel_spmd(nc, in_maps, list(range(N_CORES)), **kwargs)


def kernel(**inputs) -> np.ndarray:
    res = run(inputs)
    return assemble(res.results)


if __name__ == "__main__":
    nc = build_program()
    print("program built ok")
